# revision 25
# baseline (speedup 1.0000x reference)
"""Trainium2 Bass kernel for nn_MultiHeadAttention_62878321214362.

Problem: B=2, S=2048, D=1024, H=16 heads, DK=64, fp32, mask=all-ones.
  out = softmax((q@Wq.T+bq)(k@Wk.T+bk).T / 8) @ (v@Wv.T+bv) @ Wo.T + bo

Sharding (8 cores): core c -> batch b=c//4, head-group g=c%4 (4 heads each).
Each core computes a partial out-projection y_c = attn_out_g @ Wo[:, g-slice].T;
host sums the 4 partials per batch (the "all-reduce") and adds bo.

Math simplifications (exact up to fp rounding):
  - bk: adds a per-query constant to scores -> softmax-invariant -> dropped.
  - bv: softmax rows sum to 1, so attn@(vh + 1*bv) = attn@vh + 1*bv; the
    1*bv term is folded into the host-side constant: bo + bv @ Wo.T.
  - bq: kept (applied on device as per-partition bias in the transposed
    projection layout).

Device layout (per core), everything "transposed" so no on-chip transposes:
  qhT, khT: [hd=256, S] = W_slice @ x.T   (hd on partitions, 2 tiles of 128)
  vh:       [S, hd]  natural layout, with an appended ones-column per head
            (row 64 of the attnV output then holds the softmax denominators)
  scoresT:  [kpos, q] = khT.T @ qhT  per head
  expT = exp(scoresT/8);  outT[65, q] += vh_ext[kpos].T @ expT[kpos]
  normalize outT rows 0:64 by broadcast(1/row64) (K=1 ones matmul broadcast)
  y[s, :]  = aoT.T @ WoT_slice    (partial bf16; host sums over 4 cores)

Matmuls/storage run in bf16 with fp32 PSUM accumulation.

Performance structure (measured on this axon-relay TRN2 rig):
 - PE streams ~210-244 ns per N=512 matmul; tile_position row-pairing gives
   NO concurrency here, and DMA sustains ~275 GB/s/core. The kernel is
   PE/exp-bound.
 - softmax exp (16.8M elems/core) is split across TWO engines run in
   parallel: ACT does true Exp; DVE computes a Schraudolph exp (bf16 bits =
   round(t*128/ln2 + 127*128-c) via one fused tensor_scalar writing int16
   into a bf16-aliased tile). ~2% elementwise error on ~half the softmax
   weights -> end-to-end rel err ~7e-3 (gate 2e-2).
 - phase 2 runs per quadrant as [stream scores->exp into 34 SBUF et tiles]
   then a dependency-free 64-matmul attnV burst; one 8-bank PSUM ring is
   shared by score tiles and attnV accumulators, and the PE-side normalize
   (ones-matmul broadcast + mul) is deferred into the next quadrant's score
   stream so the in-order PE never waits on the reciprocal chain.
 - phase 1 streams x through an 8-deep shared DMA ring (hides the ~3.7us
   per-DMA latency); evacuations are split ACT/DVE.
"""

import numpy as np

B, S, D, H = 2, 2048, 1024, 16
DK = D // H          # 64
HPC = 4              # heads per core
HD = HPC * DK        # 256 per-core head dims
NCORES = 8
KT = D // 128        # 8 k-tiles for projections
ST = S // 128        # 16 s-tiles
SCALE = 1.0 / np.sqrt(np.float32(DK))

ATT_DT = "bf16"   # "bf16" | "f32r"  matmul/storage dtype for x, W, attention
_cache = {}


def _build(n_reps=1, hw_loop=0, loop_phases=(1, 2, 3), merged=False):
    import concourse.bacc as bacc
    import concourse.mybir as mybir
    import concourse.tile as tile

    F32 = mybir.dt.float32
    F32R = mybir.dt.float32r
    DT = mybir.dt.bfloat16 if ATT_DT == "bf16" else F32R

    nc = bacc.Bacc("TRN2", target_bir_lowering=False, debug=False,
                   num_devices=NCORES)

    xq = nc.dram_tensor("xq", [D, S], DT, kind="ExternalInput").ap()
    xk = nc.dram_tensor("xk", [D, S], DT, kind="ExternalInput").ap()
    xv = nc.dram_tensor("xv", [D, S], DT, kind="ExternalInput").ap()
    wq = nc.dram_tensor("wq", [D, HD], DT, kind="ExternalInput").ap()
    wk = nc.dram_tensor("wk", [D, HD], DT, kind="ExternalInput").ap()
    wv = nc.dram_tensor("wv", [D, HD], DT, kind="ExternalInput").ap()
    wo = nc.dram_tensor("wo", [HD, D], DT, kind="ExternalInput").ap()
    bq = nc.dram_tensor("bq", [128, 2], F32, kind="ExternalInput").ap()
    cst = nc.dram_tensor("cst", [128, 64], DT, kind="ExternalInput").ap()
    zc = nc.dram_tensor("zc", [1, 640], DT, kind="ExternalInput").ap()
    y = nc.dram_tensor("y", [S, D], DT, kind="ExternalOutput").ap()

    with tile.TileContext(nc) as tc:
        with (
            tc.tile_pool(name="pers", bufs=1) as pers,
            tc.tile_pool(name="stream", bufs=8) as stream,
            tc.tile_pool(name="expp", bufs=34) as expp,
            tc.tile_pool(name="small", bufs=2) as small,
            tc.tile_pool(name="ysb", bufs=4) as ysb,
        ):
            # ---- persistent SBUF tiles ----
            wq_sb = pers.tile([128, KT, HD], DT, tag="wq")
            wk_sb = pers.tile([128, KT, HD], DT, tag="wk")
            wv_sb = pers.tile([128, KT, HD], DT, tag="wv")
            wo_sb = pers.tile([128, 2, D], DT, tag="wo")
            bq_sb = pers.tile([128, 2], F32, tag="bq")
            qhT = pers.tile([128, 2, S], DT, tag="qhT")
            khT = pers.tile([128, 2, S], DT, tag="khT")
            vh = pers.tile([128, ST, HPC, DK + 1], DT, tag="vh")
            aoT = pers.tile([128, 2, S], DT, tag="aoT")
            ones64 = pers.tile([33, 64], DT, tag="ones64")
            zrow = pers.tile([1, 640], DT, tag="zrow")

            nc.sync.dma_start(wq_sb[:], wq.rearrange("(t p) n -> p t n", p=128))
            nc.sync.dma_start(wk_sb[:], wk.rearrange("(t p) n -> p t n", p=128))
            nc.sync.dma_start(wv_sb[:], wv.rearrange("(t p) n -> p t n", p=128))
            nc.sync.dma_start(wo_sb[:], wo.rearrange("(t p) n -> p t n", p=128))
            nc.sync.dma_start(bq_sb[:], bq[:])
            nc.sync.dma_start(ones64[:], cst[0:33, :])
            nc.sync.dma_start(zrow[:], zc[:])
            nc.sync.dma_start(
                vh[:, :, :, DK:DK + 1],
                cst.rearrange("p (a b c) -> p a b c", a=ST, b=HPC))

            import contextlib

            def loop_ctx(ph=0):
                on = hw_loop and not merged and (ph in loop_phases)
                return tc.For_i(0, hw_loop, 1) if on else contextlib.nullcontext()

            def outer_ctx():
                return (tc.For_i(0, hw_loop, 1) if (hw_loop and merged)
                        else contextlib.nullcontext())

            with outer_ctx():
              for rep in range(n_reps):
                # ================= Phase 1: projections =================
                with tc.tile_pool(name="pp", bufs=2, space="PSUM") as pp, loop_ctx(1):
                    # vh natural layout: [s, hd] per s-tile, 8 s-tiles per psum slot
                    # vh accumulation groups are 256 wide = half a PSUM bank, so a
                    # start=True would clear its bank-neighbor group: zero the
                    # banks once with a start=True matmul, then accumulate with
                    # start=False only.
                    pv = [pp.tile([128, S], F32, tag="pp", name=f"pv{i}") for i in range(2)]
                    for i in range(2):
                        for bank in range(4):
                            nc.tensor.matmul(
                                pv[i][:, bank * 512:(bank + 1) * 512],
                                zrow[0:1, 0:128],
                                zrow[0:1, 128:640],
                                start=True, stop=True,
                            )
                    for kt in range(KT):
                        xt = stream.tile([128, S], DT, tag="x", name=f"xv{kt}")
                        nc.sync.dma_start(xt[:], xv[kt * 128:(kt + 1) * 128, :])
                        for st in range(ST):
                            nc.tensor.matmul(
                                pv[st // 8][:, (st % 8) * HD:(st % 8 + 1) * HD],
                                xt[:, st * 128:(st + 1) * 128],
                                wv_sb[:, kt, :],
                                start=False, stop=(kt == KT - 1),
                                skip_group_check=True,
                            )
                    # evacuations split ACT/DVE so the psum ring frees faster
                    for half in range(2):
                        src = pv[half][:].rearrange("p (s h d) -> p s h d", s=8, h=HPC)
                        dst = vh[:, half * 8:(half + 1) * 8, :, 0:DK]
                        if half == 0:
                            nc.scalar.activation(
                                dst, src, mybir.ActivationFunctionType.Copy)
                        else:
                            nc.vector.tensor_copy(dst, src)

                    # qhT[mt] = Wq_sl.T @ q^T   (and +bq at evacuation)
                    pq = [pp.tile([128, S], F32, tag="pp", name=f"pq{i}") for i in range(2)]
                    for kt in range(KT):
                        xt = stream.tile([128, S], DT, tag="x", name=f"xq{kt}")
                        nc.sync.dma_start(xt[:], xq[kt * 128:(kt + 1) * 128, :])
                        for mt in range(2):
                            for c in range(4):
                                nc.tensor.matmul(
                                    pq[mt][:, c * 512:(c + 1) * 512],
                                    wq_sb[:, kt, mt * 128:(mt + 1) * 128],
                                    xt[:, c * 512:(c + 1) * 512],
                                    start=(kt == 0), stop=(kt == KT - 1),
                                )
                    for mt in range(2):
                        nc.vector.tensor_scalar_add(
                            qhT[:, mt, :], pq[mt][:], bq_sb[:, mt:mt + 1])

                    pk = [pp.tile([128, S], F32, tag="pp", name=f"pk{i}") for i in range(2)]
                    for kt in range(KT):
                        xt = stream.tile([128, S], DT, tag="x", name=f"xk{kt}")
                        nc.sync.dma_start(xt[:], xk[kt * 128:(kt + 1) * 128, :])
                        for mt in range(2):
                            for c in range(4):
                                nc.tensor.matmul(
                                    pk[mt][:, c * 512:(c + 1) * 512],
                                    wk_sb[:, kt, mt * 128:(mt + 1) * 128],
                                    xt[:, c * 512:(c + 1) * 512],
                                    start=(kt == 0), stop=(kt == KT - 1),
                                )
                    for mt in range(2):
                        nc.scalar.activation(
                            khT[:, mt, :], pk[mt][:],
                            mybir.ActivationFunctionType.Copy)

                # ================= Phase 2: attention =================
                with (
                    tc.tile_pool(name="ps", bufs=4, space="PSUM") as ps,
                    loop_ctx(2),
                ):
                    # Head-PAIR processing: the two heads of a pair live at
                    # base_partition 0 and 64 of the same khT tile, so their
                    # K=64 score matmuls target disjoint PE row-groups
                    # (tile_position auto-derives from base_partition) and run
                    # CONCURRENTLY in the array. attnV for unit u is emitted
                    # DELAY kp-units late so its exp is done when PE reaches it.
                    #
                    # exp is SPLIT across engines so neither saturates: head
                    # e=0 gets true exp on ACT; head e=1 gets a Schraudolph
                    # exp on DVE: bf16 bits = round(t*(128/ln2) + (127*128-c)),
                    # computed as one fused tensor_scalar writing int16 into a
                    # bf16-aliased tile. ~2% per-element error on half the
                    # softmax weights; end-to-end rel err ~6e-3 (gate 2e-2).
                    # v3: per quadrant, stream [scores -> exp] for ALL 16 kp
                    # into SBUF-resident et tiles (exp on ACT for head 0, DVE
                    # Schraudolph for head 1, concurrently), THEN run the 64
                    # attnV matmuls as one dependency-free burst. This removes
                    # the per-kp PE stalls of the interleaved pipeline (PE is
                    # in-order: any attnV wait blocked later scores too).
                    # The pb matmuls + final muls of quadrant q are DEFERRED
                    # into quadrant q+1's scores stream: PE is in-order, so
                    # emitting them right after attnV(q) would stall PE on the
                    # sums->recip->recr chain (~3.5us) before scores(q+1).
                    DVE_KP = 15   # kp tiles of head e=1 handled by DVE-exp
                    A_DVE = float(SCALE * 128.0 / np.log(2.0))
                    B_DVE = float(127.0 * 128.0 - 5.58)
                    pending_norm = [None]
                    for mt in range(2):
                        for qh in range(2):
                            q0 = qh * 1024
                            etq = []
                            for kp in range(ST):
                                pscs = [ps.tile([128, 1024], F32, tag="sc",
                                                name=f"psc{mt}_{qh}_{kp}_{e}")
                                        for e in range(2)]
                                for e in range(2):
                                    p0 = e * 64
                                    for c in range(2):
                                        nc.tensor.matmul(
                                            pscs[e][:, c * 512:(c + 1) * 512],
                                            khT[p0:p0 + 64, mt, kp * 128:(kp + 1) * 128],
                                            qhT[p0:p0 + 64, mt, q0 + c * 512:q0 + (c + 1) * 512],
                                            start=True, stop=True,
                                        )
                                ets = []
                                for e in range(2):
                                    et = expp.tile([128, 1024], DT, tag="expT",
                                                   name=f"et{mt}_{qh}_{kp}_{e}")
                                    if e == 1 and kp < DVE_KP:
                                        nc.vector.tensor_scalar(
                                            et[:].bitcast(mybir.dt.int16),
                                            pscs[e][:],
                                            A_DVE, B_DVE,
                                            mybir.AluOpType.mult,
                                            mybir.AluOpType.add)
                                    else:
                                        nc.scalar.activation(
                                            et[:], pscs[e][:],
                                            mybir.ActivationFunctionType.Exp,
                                            scale=float(SCALE))
                                    ets.append(et)
                                etq.append(ets)
                                if kp == 1 and pending_norm[0] is not None:
                                    pending_norm[0]()
                                    pending_norm[0] = None
                            pouts = [ps.tile([65, 1024], F32, tag="sc",
                                             name=f"pout{mt}_{qh}_{e}")
                                     for e in range(2)]
                            for kp in range(ST):
                                for e in range(2):
                                    for c in range(2):
                                        nc.tensor.matmul(
                                            pouts[e][:, c * 512:(c + 1) * 512],
                                            vh[:, kp, 2 * mt + e, :],
                                            etq[kp][e][:, c * 512:(c + 1) * 512],
                                            start=(kp == 0), stop=(kp == ST - 1),
                                        )
                            # normalize rows 0:64 by 1/row64 (bcast via K=1
                            # ones matmul). Both heads' sum rows staged at
                            # partitions 0 and 32 (32-aligned for the K=1
                            # matmul rhs), one batched reciprocal; the evac
                            # copies run now (ACT/DVE), the PE-side pb matmuls
                            # + muls are deferred (see above).
                            sums = small.tile([33, 1024], F32, tag="sums")
                            nc.scalar.activation(
                                sums[0:1, :], pouts[0][64:65, :],
                                mybir.ActivationFunctionType.Copy)
                            nc.scalar.activation(
                                sums[32:33, :], pouts[1][64:65, :],
                                mybir.ActivationFunctionType.Copy)
                            recf = small.tile([33, 1024], F32, tag="recf")
                            nc.vector.reciprocal_approx_fast(
                                out=recf[:], in_=sums[:])
                            recr = small.tile([33, 1024], DT, tag="recr")
                            nc.gpsimd.tensor_copy(recr[:], recf[:])
                            dests = []
                            for e in range(2):
                                p0 = e * 64
                                dest = aoT[p0:p0 + 64, mt, q0:q0 + 1024]
                                if e == 0:
                                    nc.scalar.activation(
                                        dest, pouts[e][0:64, :],
                                        mybir.ActivationFunctionType.Copy)
                                else:
                                    nc.vector.tensor_copy(dest, pouts[e][0:64, :])
                                dests.append(dest)

                            def mk_norm(mt=mt, qh=qh, recr=recr, dests=dests):
                                def norm():
                                    for e in range(2):
                                        for c in range(2):
                                            pb = ps.tile(
                                                [64, 512], F32, tag="sc",
                                                name=f"pb{mt}_{qh}_{e}_{c}")
                                            nc.tensor.matmul(
                                                pb[:],
                                                ones64[32 * e:32 * e + 1, :],
                                                recr[32 * e:32 * e + 1,
                                                     c * 512:(c + 1) * 512],
                                                start=True, stop=True,
                                            )
                                            nc.vector.tensor_mul(
                                                dests[e][:, c * 512:(c + 1) * 512],
                                                dests[e][:, c * 512:(c + 1) * 512],
                                                pb[:])
                                return norm

                            pending_norm[0] = mk_norm()
                    pending_norm[0]()
                    pending_norm[0] = None

                # ================= Phase 3: output projection =================
                # y partials are written bf16 (host sums in fp32); PSUM
                # evacuation alternates ACT/DVE so neither engine gates.
                with tc.tile_pool(name="py", bufs=3, space="PSUM") as py, loop_ctx(3):
                    for st in range(ST):
                        pyt = py.tile([128, 1024], F32, tag="py")
                        # kt2 outer: the aoT weight tile is shared by both nh
                        # chunks, halving Ldweights traffic
                        for kt2 in range(2):
                            for nh in range(2):
                                nc.tensor.matmul(
                                    pyt[:, nh * 512:(nh + 1) * 512],
                                    aoT[:, kt2, st * 128:(st + 1) * 128],
                                    wo_sb[:, kt2, nh * 512:(nh + 1) * 512],
                                    start=(kt2 == 0), stop=(kt2 == 1),
                                )
                        yt = ysb.tile([128, 1024], DT, tag="y")
                        if st % 2 == 0:
                            nc.scalar.activation(
                                yt[:], pyt[:], mybir.ActivationFunctionType.Copy)
                        else:
                            nc.vector.tensor_copy(yt[:], pyt[:])
                        nc.sync.dma_start(y[st * 128:(st + 1) * 128, :], yt[:])

    nc.compile()
    return nc


def make_in_maps(q, k, v, mask, Wq, bq, Wk, bk, Wv, bv, Wo, bo):
    q = np.asarray(q, dtype=np.float32)
    k = np.asarray(k, dtype=np.float32)
    v = np.asarray(v, dtype=np.float32)
    Wq, Wk, Wv, Wo = (np.asarray(w, dtype=np.float32) for w in (Wq, Wk, Wv, Wo))
    bq, bv, bo = (np.asarray(x, dtype=np.float32) for x in (bq, bv, bo))

    if ATT_DT == "bf16":
        import ml_dtypes
        ndt = ml_dtypes.bfloat16
    else:
        ndt = np.float32
    WqT, WkT, WvT, WoT = Wq.T, Wk.T, Wv.T, Wo.T
    xT = {b: {} for b in range(B)}
    for b in range(B):
        xT[b]["q"] = np.ascontiguousarray(q[b].T.astype(ndt))
        xT[b]["k"] = np.ascontiguousarray(k[b].T.astype(ndt))
        xT[b]["v"] = np.ascontiguousarray(v[b].T.astype(ndt))

    in_maps = []
    for c in range(NCORES):
        b, g = divmod(c, 4)
        hs = g * HD
        in_maps.append({
            "xq": xT[b]["q"],
            "xk": xT[b]["k"],
            "xv": xT[b]["v"],
            "wq": np.ascontiguousarray(WqT[:, hs:hs + HD].astype(ndt)),
            "wk": np.ascontiguousarray(WkT[:, hs:hs + HD].astype(ndt)),
            "wv": np.ascontiguousarray(WvT[:, hs:hs + HD].astype(ndt)),
            "wo": np.ascontiguousarray(WoT[hs:hs + HD, :].astype(ndt)),
            "bq": np.ascontiguousarray(bq[hs:hs + HD].reshape(2, 128).T),
            "cst": np.ones((128, 64), dtype=ndt),
            "zc": np.zeros((1, 640), dtype=ndt),
        })
    return in_maps


def kernel(q, k, v, mask, Wq, bq, Wk, bk, Wv, bv, Wo, bo):
    import os
    # NTFF tracing is unavailable under this axon relay (antenv.axon_hooks
    # missing); make sure an inherited BASS_TRACE can't crash the run.
    os.environ["BASS_NEVER_TRACE"] = "1"
    from concourse.bass_utils import run_bass_kernel_spmd

    if "nc" not in _cache:
        _cache["nc"] = _build()
    nc = _cache["nc"]

    in_maps = make_in_maps(q, k, v, mask, Wq, bq, Wk, bk, Wv, bv, Wo, bo)
    Wo = np.asarray(Wo, dtype=np.float32)
    bv = np.asarray(bv, dtype=np.float32)
    bo = np.asarray(bo, dtype=np.float32)

    res = run_bass_kernel_spmd(nc, in_maps, core_ids=list(range(NCORES)))
    _cache["last_results"] = res

    const = (bo + bv @ Wo.T).astype(np.float32)   # folded bv + bo correction
    out = np.empty((B, S, D), dtype=np.float32)
    for b in range(B):
        acc = res.results[4 * b]["y"].astype(np.float32).copy()
        for g in range(1, 4):
            acc += res.results[4 * b + g]["y"]
        out[b] = acc + const
    return out

